# revision 4
# baseline (speedup 1.0000x reference)
"""Trainium2 Bass kernel for top-2 MoE (B=8192, D=1024, E=8, F=1024).

Sharding: data-parallel over the batch across 8 NeuronCores. Each core gets
1024 tokens and the full (replicated) weights; no collectives are needed.
The host transposes each x shard once (layout prep) so the contraction dim
(d) lands on SBUF partitions.

Per-core plan:
  1. gating logits via fp32 matmuls (exact enough for stable top-2 selection)
  2. top-2 + softmax with the DVE max8 instruction + ACT sigmoid
  3. dense expert MLP in fp32r (full PE speed for 4-byte floats):
       hT[f, t] = gelu(w1[e].T @ xT + b1)    (w1 natural layout is lhsT)
       y[t, g]  = hT.T @ w2[e]               (hT is lhsT, w2 natural is rhs)
  4. out[t] += gate[t, e] * y[t] via fused scalar_tensor_tensor
     (b2 terms folded in as the tiny matmul gw @ b2)
"""

import os
import sys
import types

import numpy as np

_OPT_REPO = "/opt/trn_rl_repo"
if os.path.isdir(_OPT_REPO) and _OPT_REPO not in sys.path:
    sys.path.append(_OPT_REPO)

import concourse.mybir as mybir
import concourse.tile as tile
from concourse import bacc
from concourse.bass_utils import run_bass_kernel_spmd
from concourse.masks import make_identity

P = 128
B = 8192
T = 1024  # tokens per core
D = 1024
E = 8
F = 1024
NDC = D // P
NFC = F // P
NTC = T // P
TH = 512  # free-dim tile (fp32 moving operand max)

f32 = mybir.dt.float32
f32r = mybir.dt.float32r
ALU = mybir.AluOpType
ACT = mybir.ActivationFunctionType


def _r(ap):
    return ap.bitcast(f32r)


def build_moe():
    nc = bacc.Bacc("TRN2", target_bir_lowering=False, debug=False)

    xt = nc.dram_tensor("xt", [D, T], f32, kind="ExternalInput")
    gatew = nc.dram_tensor("gatew", [D, E], f32, kind="ExternalInput")
    w1 = nc.dram_tensor("w1", [E, D, F], f32, kind="ExternalInput")
    b1 = nc.dram_tensor("b1", [E, F], f32, kind="ExternalInput")
    w2 = nc.dram_tensor("w2", [E, F, F], f32, kind="ExternalInput")
    b2 = nc.dram_tensor("b2", [E, F], f32, kind="ExternalInput")
    out = nc.dram_tensor("out", [T, F], f32, kind="ExternalOutput")
    gw = nc.dram_tensor("gw", [T, E], f32, kind="ExternalOutput")

    with tile.TileContext(nc) as tc:
        with (
            tc.tile_pool(name="xt", bufs=1) as xt_pool,
            tc.tile_pool(name="const", bufs=1) as const_pool,
            tc.tile_pool(name="gate", bufs=2) as gate_pool,
            tc.tile_pool(name="gwsb", bufs=1) as gw_pool,
            tc.tile_pool(name="acc", bufs=1) as acc_pool,
            tc.tile_pool(name="w1", bufs=1) as w1_pool,
            tc.tile_pool(name="w2", bufs=1) as w2_pool,
            tc.tile_pool(name="b1", bufs=2) as b1_pool,
            tc.tile_pool(name="ht", bufs=9) as h_pool,
        ):
            # ---- persistent loads ----
            xt_sb = [xt_pool.tile([P, T], f32r, tag=f"xt{dc}", name=f"xt_sb{dc}") for dc in range(NDC)]
            for dc in range(NDC):
                nc.gpsimd.dma_start(xt_sb[dc][:], xt[dc * P : (dc + 1) * P, :])

            gatew_sb = const_pool.tile([P, NDC * E], f32)
            for dc in range(NDC):
                nc.sync.dma_start(
                    gatew_sb[:, dc * E : (dc + 1) * E],
                    gatew[dc * P : (dc + 1) * P, :],
                )
            b2_sb = const_pool.tile([E, F], f32r)
            nc.gpsimd.dma_start(b2_sb[:], b2[:, :])
            identity = const_pool.tile([P, P], f32)
            make_identity(nc, identity[:])

            gw_sb = [gw_pool.tile([P, E], f32, tag=f"gw{t}", name=f"gw_sb{t}") for t in range(NTC)]
            out_acc = [acc_pool.tile([P, F], f32, tag=f"acc{t}", name=f"out_acc{t}") for t in range(NTC)]

            # ---- gating + out_acc init (b2 mix) ----
            with tc.tile_pool(name="plog", bufs=1, space="PSUM") as plog_pool:
                plogs = [
                    plog_pool.tile([P, E], f32, tag=f"plog{t}", name=f"plog{t}")
                    for t in range(NTC)
                ]
                with tc.tile_pool(name="xtg", bufs=2) as xtg_pool:
                    for dc in range(NDC):
                        xtg = xtg_pool.tile([P, T], f32)
                        nc.sync.dma_start(xtg[:], xt[dc * P : (dc + 1) * P, :])
                        for t in range(NTC):
                            nc.tensor.matmul(
                                plogs[t][:],
                                lhsT=xtg[:, t * P : (t + 1) * P],
                                rhs=gatew_sb[:, dc * E : (dc + 1) * E],
                                start=(dc == 0),
                                stop=(dc == NDC - 1),
                            )
            with (
                tc.tile_pool(name="ptr", bufs=2, space="PSUM") as ptr_pool,
                tc.tile_pool(name="pb2", bufs=2, space="PSUM") as pb2_pool,
            ):
                for t in range(NTC):
                    tsl = slice(t * P, (t + 1) * P)
                    logits = gate_pool.tile([P, E], f32)
                    nc.vector.tensor_copy(logits[:], plogs[t][:])
                    m8 = gate_pool.tile([P, 8], f32)
                    nc.vector.max(m8[:], logits[:])
                    diff = gate_pool.tile([P, 1], f32)
                    nc.vector.tensor_sub(diff[:], m8[:, 1:2], m8[:, 0:1])
                    p2 = gate_pool.tile([P, 1], f32)
                    nc.scalar.activation(p2[:], diff[:], ACT.Sigmoid)
                    p1 = gate_pool.tile([P, 1], f32)
                    nc.vector.tensor_scalar(
                        p1[:], p2[:], -1.0, 1.0, op0=ALU.mult, op1=ALU.add
                    )
                    eq1 = gate_pool.tile([P, E], f32)
                    nc.vector.tensor_scalar(
                        eq1[:], logits[:], m8[:, 0:1], None, op0=ALU.is_equal
                    )
                    eq2 = gate_pool.tile([P, E], f32)
                    nc.vector.tensor_scalar(
                        eq2[:], logits[:], m8[:, 1:2], None, op0=ALU.is_equal
                    )
                    nc.vector.tensor_scalar_mul(eq1[:], eq1[:], p1[:, 0:1])
                    nc.vector.tensor_scalar_mul(eq2[:], eq2[:], p2[:, 0:1])
                    nc.vector.tensor_add(gw_sb[t][:], eq1[:], eq2[:])
                    nc.sync.dma_start(gw[tsl, :], gw_sb[t][:])

                    # out_acc[t] = gw[t] @ b2  (tiny K=8 matmul via transpose)
                    ptr = ptr_pool.tile([E, P], f32)
                    nc.tensor.transpose(ptr[:], gw_sb[t][:], identity[:])
                    gwt = gate_pool.tile([E, P], f32r)
                    nc.vector.tensor_copy(gwt[:], ptr[:])
                    for gh in range(2):
                        gsl = slice(gh * TH, (gh + 1) * TH)
                        pb = pb2_pool.tile([P, TH], f32)
                        nc.tensor.matmul(
                            pb[:],
                            lhsT=gwt[:],
                            rhs=b2_sb[:, gsl],
                            start=True,
                            stop=True,
                        )
                        nc.vector.tensor_copy(out_acc[t][:, gsl], pb[:])

            # ---- dense expert MLP ----
            with (
                tc.tile_pool(name="ph", bufs=3, space="PSUM") as ph_pool,
                tc.tile_pool(name="py", bufs=3, space="PSUM") as py_pool,
            ):
                for e in range(E):
                    b1_sb = b1_pool.tile([P, NFC], f32)
                    for fc in range(NFC):
                        nc.sync.dma_start(
                            b1_sb[:, fc : fc + 1],
                            b1[e, fc * P : (fc + 1) * P, None],
                        )
                    w1_sb = [
                        w1_pool.tile([P, F], f32r, tag=f"w1_{dc}", name=f"w1_sb{dc}") for dc in range(NDC)
                    ]
                    for dc in range(NDC):
                        nc.gpsimd.dma_start(w1_sb[dc][:], w1[e, dc * P : (dc + 1) * P, :])
                    w2_sb = [
                        w2_pool.tile([P, F], f32r, tag=f"w2_{fc}", name=f"w2_sb{fc}") for fc in range(NFC)
                    ]
                    for fc in range(NFC):
                        nc.gpsimd.dma_start(w2_sb[fc][:], w2[e, fc * P : (fc + 1) * P, :])

                    ht = [h_pool.tile([P, T], f32r, tag="ht", name=f"ht{fc}") for fc in range(NFC)]
                    for fc in range(NFC):
                        fsl = slice(fc * P, (fc + 1) * P)
                        for th in range(2):
                            hsl = slice(th * TH, (th + 1) * TH)
                            ph = ph_pool.tile([P, TH], f32)
                            for dc in range(NDC):
                                nc.tensor.matmul(
                                    ph[:],
                                    lhsT=w1_sb[dc][:, fsl],
                                    rhs=xt_sb[dc][:, hsl],
                                    start=(dc == 0),
                                    stop=(dc == NDC - 1),
                                )
                            nc.scalar.activation(
                                ht[fc][:, hsl],
                                ph[:],
                                ACT.Gelu,
                                bias=b1_sb[:, fc : fc + 1],
                            )
                    for t in range(NTC):
                        tsl = slice(t * P, (t + 1) * P)
                        for gh in range(2):
                            gsl = slice(gh * TH, (gh + 1) * TH)
                            py = py_pool.tile([P, TH], f32)
                            for fc in range(NFC):
                                nc.tensor.matmul(
                                    py[:],
                                    lhsT=ht[fc][:, tsl],
                                    rhs=w2_sb[fc][:, gsl],
                                    start=(fc == 0),
                                    stop=(fc == NFC - 1),
                                )
                            nc.vector.scalar_tensor_tensor(
                                out=out_acc[t][:, gsl],
                                in0=py[:],
                                scalar=gw_sb[t][:, e : e + 1],
                                in1=out_acc[t][:, gsl],
                                op0=ALU.mult,
                                op1=ALU.add,
                            )

            for t in range(NTC):
                nc.sync.dma_start(out[t * P : (t + 1) * P, :], out_acc[t][:])

    nc.compile()
    return nc


_NC = None


def _get_nc():
    global _NC
    if _NC is None:
        _NC = build_moe()
    return _NC


def _maybe_enable_trace():
    """Register the NTFF profile hook (missing antenv.axon_hooks shim)."""
    try:
        import antenv

        if "antenv.axon_hooks" not in sys.modules:
            hooks = types.ModuleType("antenv.axon_hooks")
            hooks._hook = None
            hooks.set_axon_ntff_profile_hook = lambda h: setattr(hooks, "_hook", h)
            hooks.get_axon_ntff_profile_hook = lambda: hooks._hook
            sys.modules["antenv.axon_hooks"] = hooks
            antenv.axon_hooks = hooks
            from trn_agent_boot.trn_boot import _ntff_profile_via_ctypes

            hooks.set_axon_ntff_profile_hook(
                _ntff_profile_via_ctypes("/opt/axon/libaxon_pjrt.so")
            )
        return True
    except Exception:
        return False


LAST_EXEC_TIME_NS = None


def kernel(x, gate_w, w1, b1, w2, b2):
    global LAST_EXEC_TIME_NS
    x = np.ascontiguousarray(np.asarray(x, dtype=np.float32))
    gate_w = np.ascontiguousarray(np.asarray(gate_w, dtype=np.float32))
    w1 = np.ascontiguousarray(np.asarray(w1, dtype=np.float32))
    b1 = np.ascontiguousarray(np.asarray(b1, dtype=np.float32))
    w2 = np.ascontiguousarray(np.asarray(w2, dtype=np.float32))
    b2 = np.ascontiguousarray(np.asarray(b2, dtype=np.float32))

    trace = bool(os.environ.get("BASS_MOE_TRACE"))
    if trace:
        trace = _maybe_enable_trace()

    nc = _get_nc()
    in_maps = []
    for c in range(8):
        xs = x[c * T : (c + 1) * T]
        in_maps.append(
            {
                "xt": np.ascontiguousarray(xs.T),
                "gatew": gate_w,
                "w1": w1,
                "b1": b1,
                "w2": w2,
                "b2": b2,
            }
        )
    res = run_bass_kernel_spmd(nc, in_maps, list(range(8)), trace=trace)
    LAST_EXEC_TIME_NS = res.exec_time_ns
    out = np.concatenate([res.results[c]["out"] for c in range(8)], axis=0)
    gws = np.concatenate([res.results[c]["gw"] for c in range(8)], axis=0)
    return out, gws


# revision 6
# speedup vs baseline: 1.0674x; 1.0674x over previous
"""Trainium2 Bass kernel for top-2 MoE (B=8192, D=1024, E=8, F=1024).

Sharding: data-parallel over the batch across 8 NeuronCores. Each core gets
1024 tokens and the full (replicated) weights; no collectives are needed.
The host transposes each x shard once (layout prep) so the contraction dim
(d) lands on SBUF partitions.

Per-core plan:
  1. gating logits via fp32 matmuls (exact enough for stable top-2 selection)
  2. top-2 + softmax with the DVE max8 instruction + ACT sigmoid
  3. dense expert MLP in fp32r (full PE speed for 4-byte floats):
       hT[f, t] = gelu(w1[e].T @ xT + b1)    (w1 natural layout is lhsT)
       y[t, g]  = hT.T @ w2[e]               (hT is lhsT, w2 natural is rhs)
  4. out[t] += gate[t, e] * y[t] via fused scalar_tensor_tensor
     (b2 terms folded in as the tiny matmul gw @ b2)
"""

import os
import sys
import types

import numpy as np

_OPT_REPO = "/opt/trn_rl_repo"
if os.path.isdir(_OPT_REPO) and _OPT_REPO not in sys.path:
    sys.path.append(_OPT_REPO)

import concourse.mybir as mybir
import concourse.tile as tile
from concourse import bacc
from concourse.bass_utils import run_bass_kernel_spmd
from concourse.masks import make_identity

P = 128
B = 8192
T = 1024  # tokens per core
D = 1024
E = 8
F = 1024
NDC = D // P
NFC = F // P
NTC = T // P
TH = 512  # free-dim tile (fp32 moving operand max)

f32 = mybir.dt.float32
f32r = mybir.dt.float32r
bf16 = mybir.dt.bfloat16
ALU = mybir.AluOpType
ACT = mybir.ActivationFunctionType


def _r(ap):
    return ap.bitcast(f32r)


def build_moe():
    nc = bacc.Bacc("TRN2", target_bir_lowering=False, debug=False)

    xt = nc.dram_tensor("xt", [D, T], f32, kind="ExternalInput")
    gatew = nc.dram_tensor("gatew", [D, E], f32, kind="ExternalInput")
    w1 = nc.dram_tensor("w1", [E, D, F], f32, kind="ExternalInput")
    b1 = nc.dram_tensor("b1", [E, F], f32, kind="ExternalInput")
    w2 = nc.dram_tensor("w2", [E, F, F], f32, kind="ExternalInput")
    b2 = nc.dram_tensor("b2", [E, F], f32, kind="ExternalInput")
    out = nc.dram_tensor("out", [T, F], f32, kind="ExternalOutput")
    gw = nc.dram_tensor("gw", [T, E], f32, kind="ExternalOutput")

    with tile.TileContext(nc) as tc:
        with (
            tc.tile_pool(name="xt", bufs=1) as xt_pool,
            tc.tile_pool(name="const", bufs=1) as const_pool,
            tc.tile_pool(name="gate", bufs=2) as gate_pool,
            tc.tile_pool(name="gwsb", bufs=1) as gw_pool,
            tc.tile_pool(name="acc", bufs=1) as acc_pool,
            tc.tile_pool(name="w1", bufs=1) as w1_pool,
            tc.tile_pool(name="w2", bufs=1) as w2_pool,
            tc.tile_pool(name="b1", bufs=2) as b1_pool,
            tc.tile_pool(name="ht", bufs=9) as h_pool,
        ):
            # ---- persistent loads ----
            xt_sb = [xt_pool.tile([P, T], bf16, tag=f"xt{dc}", name=f"xt_sb{dc}") for dc in range(NDC)]
            for dc in range(NDC):
                nc.gpsimd.dma_start(xt_sb[dc][:], xt[dc * P : (dc + 1) * P, :])

            gatew_sb = const_pool.tile([P, NDC * E], f32)
            for dc in range(NDC):
                nc.sync.dma_start(
                    gatew_sb[:, dc * E : (dc + 1) * E],
                    gatew[dc * P : (dc + 1) * P, :],
                )
            b2_sb = const_pool.tile([E, F], bf16)
            nc.gpsimd.dma_start(b2_sb[:], b2[:, :])
            identity = const_pool.tile([P, P], f32)
            make_identity(nc, identity[:])

            gw_sb = [gw_pool.tile([P, E], f32, tag=f"gw{t}", name=f"gw_sb{t}") for t in range(NTC)]
            out_acc = [acc_pool.tile([P, F], f32, tag=f"acc{t}", name=f"out_acc{t}") for t in range(NTC)]

            # ---- gating + out_acc init (b2 mix) ----
            with tc.tile_pool(name="plog", bufs=1, space="PSUM") as plog_pool:
                plogs = [
                    plog_pool.tile([P, E], f32, tag=f"plog{t}", name=f"plog{t}")
                    for t in range(NTC)
                ]
                with tc.tile_pool(name="xtg", bufs=2) as xtg_pool:
                    for dc in range(NDC):
                        xtg = xtg_pool.tile([P, T], f32)
                        nc.sync.dma_start(xtg[:], xt[dc * P : (dc + 1) * P, :])
                        for t in range(NTC):
                            nc.tensor.matmul(
                                plogs[t][:],
                                lhsT=xtg[:, t * P : (t + 1) * P],
                                rhs=gatew_sb[:, dc * E : (dc + 1) * E],
                                start=(dc == 0),
                                stop=(dc == NDC - 1),
                            )
            with (
                tc.tile_pool(name="ptr", bufs=2, space="PSUM") as ptr_pool,
                tc.tile_pool(name="pb2", bufs=2, space="PSUM") as pb2_pool,
            ):
                for t in range(NTC):
                    tsl = slice(t * P, (t + 1) * P)
                    logits = gate_pool.tile([P, E], f32)
                    nc.vector.tensor_copy(logits[:], plogs[t][:])
                    m8 = gate_pool.tile([P, 8], f32)
                    nc.vector.max(m8[:], logits[:])
                    diff = gate_pool.tile([P, 1], f32)
                    nc.vector.tensor_sub(diff[:], m8[:, 1:2], m8[:, 0:1])
                    p2 = gate_pool.tile([P, 1], f32)
                    nc.scalar.activation(p2[:], diff[:], ACT.Sigmoid)
                    p1 = gate_pool.tile([P, 1], f32)
                    nc.vector.tensor_scalar(
                        p1[:], p2[:], -1.0, 1.0, op0=ALU.mult, op1=ALU.add
                    )
                    eq1 = gate_pool.tile([P, E], f32)
                    nc.vector.tensor_scalar(
                        eq1[:], logits[:], m8[:, 0:1], None, op0=ALU.is_equal
                    )
                    eq2 = gate_pool.tile([P, E], f32)
                    nc.vector.tensor_scalar(
                        eq2[:], logits[:], m8[:, 1:2], None, op0=ALU.is_equal
                    )
                    nc.vector.tensor_scalar_mul(eq1[:], eq1[:], p1[:, 0:1])
                    nc.vector.tensor_scalar_mul(eq2[:], eq2[:], p2[:, 0:1])
                    nc.vector.tensor_add(gw_sb[t][:], eq1[:], eq2[:])
                    nc.sync.dma_start(gw[tsl, :], gw_sb[t][:])

                    # out_acc[t] = gw[t] @ b2  (tiny K=8 matmul via transpose)
                    ptr = ptr_pool.tile([E, P], f32)
                    nc.tensor.transpose(ptr[:], gw_sb[t][:], identity[:])
                    gwt = gate_pool.tile([E, P], bf16)
                    nc.vector.tensor_copy(gwt[:], ptr[:])
                    for gh in range(2):
                        gsl = slice(gh * TH, (gh + 1) * TH)
                        pb = pb2_pool.tile([P, TH], f32)
                        nc.tensor.matmul(
                            pb[:],
                            lhsT=gwt[:],
                            rhs=b2_sb[:, gsl],
                            start=True,
                            stop=True,
                        )
                        nc.vector.tensor_copy(out_acc[t][:, gsl], pb[:])

            # ---- dense expert MLP ----
            with (
                tc.tile_pool(name="ph", bufs=3, space="PSUM") as ph_pool,
                tc.tile_pool(name="py", bufs=3, space="PSUM") as py_pool,
            ):
                for e in range(E):
                    b1_sb = b1_pool.tile([P, NFC], f32)
                    for fc in range(NFC):
                        nc.sync.dma_start(
                            b1_sb[:, fc : fc + 1],
                            b1[e, fc * P : (fc + 1) * P, None],
                        )
                    w1_sb = [
                        w1_pool.tile([P, F], bf16, tag=f"w1_{dc}", name=f"w1_sb{dc}") for dc in range(NDC)
                    ]
                    for dc in range(NDC):
                        nc.gpsimd.dma_start(w1_sb[dc][:], w1[e, dc * P : (dc + 1) * P, :])
                    w2_sb = [
                        w2_pool.tile([P, F], bf16, tag=f"w2_{fc}", name=f"w2_sb{fc}") for fc in range(NFC)
                    ]
                    for fc in range(NFC):
                        nc.gpsimd.dma_start(w2_sb[fc][:], w2[e, fc * P : (fc + 1) * P, :])

                    ht = [h_pool.tile([P, T], bf16, tag="ht", name=f"ht{fc}") for fc in range(NFC)]
                    for fc in range(NFC):
                        fsl = slice(fc * P, (fc + 1) * P)
                        for th in range(2):
                            hsl = slice(th * TH, (th + 1) * TH)
                            ph = ph_pool.tile([P, TH], f32)
                            for dc in range(NDC):
                                nc.tensor.matmul(
                                    ph[:],
                                    lhsT=w1_sb[dc][:, fsl],
                                    rhs=xt_sb[dc][:, hsl],
                                    start=(dc == 0),
                                    stop=(dc == NDC - 1),
                                )
                            nc.scalar.activation(
                                ht[fc][:, hsl],
                                ph[:],
                                ACT.Gelu,
                                bias=b1_sb[:, fc : fc + 1],
                            )
                    for t in range(NTC):
                        tsl = slice(t * P, (t + 1) * P)
                        for gh in range(2):
                            gsl = slice(gh * TH, (gh + 1) * TH)
                            py = py_pool.tile([P, TH], f32)
                            for fc in range(NFC):
                                nc.tensor.matmul(
                                    py[:],
                                    lhsT=ht[fc][:, tsl],
                                    rhs=w2_sb[fc][:, gsl],
                                    start=(fc == 0),
                                    stop=(fc == NFC - 1),
                                )
                            nc.vector.scalar_tensor_tensor(
                                out=out_acc[t][:, gsl],
                                in0=py[:],
                                scalar=gw_sb[t][:, e : e + 1],
                                in1=out_acc[t][:, gsl],
                                op0=ALU.mult,
                                op1=ALU.add,
                            )

            for t in range(NTC):
                nc.sync.dma_start(out[t * P : (t + 1) * P, :], out_acc[t][:])

    nc.compile()
    return nc


_NC = None


def _get_nc():
    global _NC
    if _NC is None:
        _NC = build_moe()
    return _NC


def _maybe_enable_trace():
    """Register the NTFF profile hook (missing antenv.axon_hooks shim)."""
    try:
        import antenv

        if "antenv.axon_hooks" not in sys.modules:
            hooks = types.ModuleType("antenv.axon_hooks")
            hooks._hook = None
            hooks.set_axon_ntff_profile_hook = lambda h: setattr(hooks, "_hook", h)
            hooks.get_axon_ntff_profile_hook = lambda: hooks._hook
            sys.modules["antenv.axon_hooks"] = hooks
            antenv.axon_hooks = hooks
            from trn_agent_boot.trn_boot import _ntff_profile_via_ctypes

            hooks.set_axon_ntff_profile_hook(
                _ntff_profile_via_ctypes("/opt/axon/libaxon_pjrt.so")
            )
        return True
    except Exception:
        return False


LAST_EXEC_TIME_NS = None
LAST_RESULT = None


def kernel(x, gate_w, w1, b1, w2, b2):
    global LAST_EXEC_TIME_NS
    x = np.ascontiguousarray(np.asarray(x, dtype=np.float32))
    gate_w = np.ascontiguousarray(np.asarray(gate_w, dtype=np.float32))
    w1 = np.ascontiguousarray(np.asarray(w1, dtype=np.float32))
    b1 = np.ascontiguousarray(np.asarray(b1, dtype=np.float32))
    w2 = np.ascontiguousarray(np.asarray(w2, dtype=np.float32))
    b2 = np.ascontiguousarray(np.asarray(b2, dtype=np.float32))

    trace = bool(os.environ.get("BASS_MOE_TRACE"))
    if trace:
        trace = _maybe_enable_trace()

    nc = _get_nc()
    in_maps = []
    for c in range(8):
        xs = x[c * T : (c + 1) * T]
        in_maps.append(
            {
                "xt": np.ascontiguousarray(xs.T),
                "gatew": gate_w,
                "w1": w1,
                "b1": b1,
                "w2": w2,
                "b2": b2,
            }
        )
    res = run_bass_kernel_spmd(nc, in_maps, list(range(8)), trace=trace)
    global LAST_RESULT
    LAST_RESULT = res
    LAST_EXEC_TIME_NS = res.exec_time_ns
    out = np.concatenate([res.results[c]["out"] for c in range(8)], axis=0)
    gws = np.concatenate([res.results[c]["gw"] for c in range(8)], axis=0)
    return out, gws
